# revision 30
# baseline (speedup 1.0000x reference)
"""TRN2 Bass kernel for nn_Block_QKA (spiking Token-QK-attention block).

Strategy
--------
Data-parallel over batch: B=16 -> 8 cores x B_local=2. Free dim per core
F = B_local * N = 2*256 = 512. All pointwise ops put channels on SBUF
partitions (C=384 -> 3 tiles of 128; hid=1536 -> 12 tiles).

LIF recurrences (6 nodes) run sequentially over T=4 with fp32 state:
    u = w + x_t                    (w = post-reset membrane, init 0)
    spike  s = [u >= 2*vth]        (since v' = 0.5*u)
    lt = 0.5*(1 - s)  in {0, 0.5}  ("inverted-half spike", exact in bf16)
    w' = u * lt  (= 0.5*u*(1-s) = v''_t, decay+reset in one mult)
Consumers of spikes read lt with host-folded weights:
    W @ s = W @ (1 - 2*lt) = rowsum(W) - (2W) @ lt
BN (inference) is folded into weights/biases on the host, so each matmul
epilogue is one fused scalar_tensor_tensor: u_next = (psum + bias) + w.

Matmul precision tiers (probed on HW):
  q,k,p,fc1 : fp32r single pass (~1e-4 rel wt error, 1 PE-cycle/row;
              measured impact: 279/6.3M outputs >1e-2, norm-rel +1e-5)
  fc2       : single-pass bf16 (~1.6e-3 rel) - feeds only the output
              residual, no threshold after it
  head-sum / expand: bf16, exact (spike values are powers of two)
Engine split (timeline-sim balanced): DVE does all PSUM-touching ops
(STT drains, z-max) + f32r spike thresholds; GPSIMD does SBUF adds and
state resets (+ t=3 spike thresholds, its idle window); ACT does t=0
epilogues and the lif4 affine. fc1/lif6 uses paired m-tiles (wide DVE
ops over [128,1024]); fc2 emission is rotated into the next timestep's
q-phase to fill the PE gap; stage0(t+1) is emitted before fc2(t) so
spikes for t+1 are ready when PE reaches them.
"""

import numpy as np
import ml_dtypes

# ---- problem constants (hardcoded per contract) ----
T, B, C, H, W_SP = 4, 16, 384, 16, 16
N = H * W_SP                # 256 spatial
HID = 1536
NH, DH = 8, 48              # heads
NCORES = 8
BL = B // NCORES            # 2 batches per core
F = BL * N                  # 512 free dim
CT = C // 128               # 3 channel tiles
HT = HID // 128             # 12 hidden tiles
NB = 3 * CT + HT + CT + 1   # bias columns
BN_EPS = 1e-5

_COMPILED = {}              # cache: built bass module across calls


def _fold_host(qw, bn_q, kw, bn_k, pw, pb, bn_p, f1w, f1b, bn1, f2w, f2b, bn2):
    """Fold BN + spike-encoding into matmul weights/biases (float64 math)."""
    def bn_parts(p):
        g, b, m, v = [p[i].astype(np.float64) for i in range(4)]
        scale = g / np.sqrt(v + BN_EPS)
        return scale, b - m * scale

    def fold(w, scale, bias0, extra_bias=None):
        w = w.astype(np.float64)
        weff = -2.0 * scale[:, None] * w
        beff = scale * (w.sum(axis=1) + (0.0 if extra_bias is None
                                         else extra_bias.astype(np.float64))) + bias0
        return weff, beff

    sq, b0q = bn_parts(bn_q)
    sk, b0k = bn_parts(bn_k)
    sp, b0p = bn_parts(bn_p)
    s1, b01 = bn_parts(bn1)
    s2, b02 = bn_parts(bn2)
    wq_eff, bq_eff = fold(qw, sq, b0q)
    wk_eff, bk_eff = fold(kw, sk, b0k)
    wp_eff, bp_eff = fold(pw, sp, b0p, pb)
    w1_eff, b1_eff = fold(f1w, s1, b01, f1b)
    w2_eff, b2_eff = fold(f2w, s2, b02, f2b)
    return (wq_eff, bq_eff, wk_eff, bk_eff, wp_eff, bp_eff,
            w1_eff, b1_eff, w2_eff, b2_eff)


def _split_hilo_pm(w_eff_T):
    """lhsT (K,M) float64 -> partition-major (128, KT, 2, M) bf16 hi/lo."""
    w32 = w_eff_T.astype(np.float32)
    hi = w32.astype(ml_dtypes.bfloat16)
    lo = (w32 - hi.astype(np.float32)).astype(ml_dtypes.bfloat16)
    K, M = w32.shape
    out = np.empty((128, K // 128, 2, M), dtype=ml_dtypes.bfloat16)
    out[:, :, 0, :] = hi.reshape(K // 128, 128, M).transpose(1, 0, 2)
    out[:, :, 1, :] = lo.reshape(K // 128, 128, M).transpose(1, 0, 2)
    return out


def _pm(w_T, dtype):
    """lhsT (K,M) -> partition-major (128, KT, M) of given dtype."""
    K, M = w_T.shape
    return np.ascontiguousarray(
        w_T.astype(dtype).reshape(K // 128, 128, M).transpose(1, 0, 2))


def _build_module():
    import concourse.bacc as bacc
    import concourse.tile as tile
    from concourse import mybir
    from concourse.alu_op_type import AluOpType as alu

    f32 = mybir.dt.float32
    f32r = mybir.dt.float32r
    bf16 = mybir.dt.bfloat16
    AF = mybir.ActivationFunctionType

    nc = bacc.Bacc("TRN2", target_bir_lowering=False, debug=False)

    # ---- DRAM I/O (all partition-major & contiguous for fast DMA) ----
    d_x = nc.dram_tensor("x", [T, 128, CT, F], f32, kind="ExternalInput").ap()
    d_wq = nc.dram_tensor("wq", [128, CT, C], f32, kind="ExternalInput").ap()
    d_wk = nc.dram_tensor("wk", [128, CT, C], f32, kind="ExternalInput").ap()
    d_wp = nc.dram_tensor("wp", [128, CT, C], f32, kind="ExternalInput").ap()
    d_w1 = nc.dram_tensor("w1", [128, CT, HID], f32, kind="ExternalInput").ap()
    d_w2 = nc.dram_tensor("w2", [128, HT, C], bf16, kind="ExternalInput").ap()
    d_msk = nc.dram_tensor("msk", [128, CT, NH], bf16, kind="ExternalInput").ap()
    d_ee = nc.dram_tensor("ee", [NH, C], bf16, kind="ExternalInput").ap()
    d_bias = nc.dram_tensor("bias", [128, NB], f32, kind="ExternalInput").ap()
    d_out = nc.dram_tensor("out", [T, 128, CT, F], f32, kind="ExternalOutput").ap()

    with tile.TileContext(nc) as tc:
        with (
            tc.tile_pool(name="const", bufs=1) as const,
            tc.tile_pool(name="state", bufs=1) as state,
            tc.tile_pool(name="xin", bufs=2) as xin,
            tc.tile_pool(name="x2p", bufs=2) as x2p,
            tc.tile_pool(name="up", bufs=5) as up,
            tc.tile_pool(name="up2", bufs=3) as up2,
            tc.tile_pool(name="spk", bufs=2) as spk,
            tc.tile_pool(name="spk6", bufs=1) as spk6,
            tc.tile_pool(name="outp", bufs=2) as outp,
            tc.tile_pool(name="ps_qk", bufs=2, space="PSUM") as ps_qk,
            tc.tile_pool(name="ps_misc", bufs=1, space="PSUM") as ps_misc,
            tc.tile_pool(name="ps_p", bufs=2, space="PSUM") as ps_p,
            tc.tile_pool(name="ps_f1", bufs=2, space="PSUM") as ps_f1,
            tc.tile_pool(name="ps_f2", bufs=1, space="PSUM") as ps_f2,
        ):
            # ---- load constants (single contiguous DMA each) ----
            t_wq = const.tile([128, CT, C], f32)
            t_wk = const.tile([128, CT, C], f32)
            t_wp = const.tile([128, CT, C], f32)
            t_w1 = const.tile([128, CT, HID], f32)
            t_w2 = const.tile([128, HT, C], bf16)
            t_msk = const.tile([128, CT, NH], bf16)
            t_ee = const.tile([128, C], bf16)
            t_bias = const.tile([128, NB], f32)
            xt0 = xin.tile([128, CT, F], f32, tag="xt")
            nc.sync.dma_start(out=xt0[:, 0, :], in_=d_x[0, :, 0, :])
            nc.sync.dma_start(out=t_bias, in_=d_bias)
            nc.sync.dma_start(out=t_wq.bitcast(f32r), in_=d_wq.bitcast(f32r))
            nc.sync.dma_start(out=xt0[:, 1, :], in_=d_x[0, :, 1, :])
            nc.sync.dma_start(out=xt0[:, 2, :], in_=d_x[0, :, 2, :])
            nc.sync.dma_start(out=t_wk.bitcast(f32r), in_=d_wk.bitcast(f32r))
            nc.sync.dma_start(out=t_msk, in_=d_msk)
            nc.sync.dma_start(out=t_ee[0:NH, :], in_=d_ee)
            nc.sync.dma_start(out=t_wp.bitcast(f32r), in_=d_wp.bitcast(f32r))
            nc.sync.dma_start(out=t_w1.bitcast(f32r), in_=d_w1.bitcast(f32r))
            nc.sync.dma_start(out=t_w2, in_=d_w2)

            def bias_ap(idx):
                return t_bias[:, idx:idx + 1]
            BQ, BK, BP, B1, B2 = 0, CT, 2 * CT, 3 * CT, 3 * CT + HT
            B4 = 3 * CT + HT + CT  # lif4 constant bias column (= DH)

            # ---- persistent LIF states (t=0 skips the add -> no init) ----
            w1s = state.tile([128, CT, F], f32)
            w2s = state.tile([128, CT, F], f32)
            w3s = state.tile([128, CT, F], f32)
            w4s = state.tile([128, F], f32)
            w5s = state.tile([128, CT, F], f32)
            w6s = state.tile([128, HT, F], f32)

            def gps_add(out, a, b):
                nc.gpsimd.tensor_tensor(out, a, b, alu.add)

            def gps_reset(wst, u, lt):
                nc.gpsimd.tensor_tensor(wst, u, lt, alu.mult)

            def stage0(t, xt):
                """lif1 -> xlt(t). Returns xlt tile."""
                xlt = spk.tile([128, CT, F], f32, tag="xlt")
                for m in range(CT):
                    if t == 0:
                        u1 = xt[:, m, :]
                    else:
                        u1 = up.tile([128, F], f32, tag="u")
                        gps_add(u1, xt[:, m, :], w1s[:, m, :])
                    nc.vector.tensor_scalar(xlt[:, m, :].bitcast(f32r), u1,
                                            2.0, 0.5, alu.is_lt, alu.mult)
                    if t < T - 1:
                        nc.vector.tensor_tensor(w1s[:, m, :], u1, xlt[:, m, :],
                                                alu.mult)
                return xlt

            pending_f2 = None
            pending_f2b = None
            pending_f2c = None
            xts = [None] * T
            xts[0] = xt0
            xlt_cur = stage0(0, xt0)

            for t in range(T):
                xt = xts[t]
                xlt = xlt_cur

                # ===== q matmuls + lif2 -> qlt =====
                qlt = spk.tile([128, CT, F], bf16, tag="qlt")
                for m in range(CT):
                    ps = ps_qk.tile([128, F], f32, tag="qk")
                    for k in range(CT):
                        nc.tensor.matmul(
                            ps, t_wq[:, k, m * 128:(m + 1) * 128].bitcast(f32r),
                            xlt[:, k, :].bitcast(f32r),
                            start=(k == 0), stop=(k == CT - 1))
                    u = up.tile([128, F], f32, tag="u")
                    if t == 0:
                        nc.scalar.activation(u, ps, AF.Identity,
                                             bias=bias_ap(BQ + m), scale=1.0)
                    else:
                        nc.vector.scalar_tensor_tensor(
                            u, ps, bias_ap(BQ + m), w2s[:, m, :], alu.add, alu.add)
                    engq = nc.gpsimd if t == T - 1 else nc.vector
                    engq.tensor_scalar(qlt[:, m, :], u, 2.0, 0.5,
                                       alu.is_lt, alu.mult)
                    if t < T - 1:
                        gps_reset(w2s[:, m, :], u, qlt[:, m, :])

                # fc2 of t-1 fills the PE gap while lif2/headsum catch up
                if pending_f2 is not None:
                    pending_f2()
                    pending_f2 = None

                # ===== attn head-sum + lif4 (hides under k matmuls) =====
                ps4 = ps_misc.tile([128, F], f32, tag="misc")
                for k in range(CT):
                    nc.tensor.matmul(ps4[0:NH, :], t_msk[:, k, :], qlt[:, k, :],
                                     start=(k == 0), stop=(k == CT - 1))
                x4 = up.tile([128, F], f32, tag="u")
                nc.scalar.activation(x4[0:NH, :], ps4[0:NH, :], AF.Identity,
                                     bias=t_bias[0:NH, B4:B4 + 1], scale=-2.0)
                if t == 0:
                    u4 = x4
                else:
                    u4 = up.tile([128, F], f32, tag="u")
                    nc.vector.tensor_tensor(u4[0:NH, :], x4[0:NH, :], w4s[0:NH, :],
                                            alu.add)
                alt = spk.tile([128, F], bf16, tag="alt")
                nc.vector.tensor_scalar(alt[0:NH, :], u4[0:NH, :], 1.0, 0.5,
                                        alu.is_lt, alu.mult)
                if t < T - 1:
                    nc.vector.tensor_tensor(w4s[0:NH, :], u4[0:NH, :], alt[0:NH, :],
                                            alu.mult)

                # ===== k matmuls + lif3 -> klt, fused expand/z =====
                klt = spk.tile([128, CT, F], bf16, tag="klt")
                z = spk.tile([128, CT, F], f32, tag="z")
                for m in range(CT):
                    ps = ps_qk.tile([128, F], f32, tag="qk")
                    for k in range(CT):
                        nc.tensor.matmul(
                            ps, t_wk[:, k, m * 128:(m + 1) * 128].bitcast(f32r),
                            xlt[:, k, :].bitcast(f32r),
                            start=(k == 0), stop=(k == CT - 1))
                    u = up.tile([128, F], f32, tag="u")
                    if t == 0:
                        nc.scalar.activation(u, ps, AF.Identity,
                                             bias=bias_ap(BK + m), scale=1.0)
                    else:
                        nc.vector.scalar_tensor_tensor(
                            u, ps, bias_ap(BK + m), w3s[:, m, :], alu.add, alu.add)
                    engk = nc.gpsimd if t == T - 1 else nc.vector
                    engk.tensor_scalar(klt[:, m, :], u, 2.0, 0.5,
                                       alu.is_lt, alu.mult)
                    if t < T - 1:
                        gps_reset(w3s[:, m, :], u, klt[:, m, :])
                    pse = ps_misc.tile([128, F], f32, tag="misc",
                                       name=f"pse_{t}_{m}")
                    nc.tensor.matmul(pse, t_ee[0:NH, m * 128:(m + 1) * 128],
                                     alt[0:NH, :], start=True, stop=True)
                    nc.vector.tensor_tensor(z[:, m, :].bitcast(f32r), pse,
                                            klt[:, m, :], alu.max)

                # fc2(t-1) part 2 fills the PE gap while z settles
                if pending_f2b is not None:
                    pending_f2b()
                    pending_f2b = None

                # ===== p matmul, x2 = psum + bp + x, lif5 =====
                x2 = x2p.tile([128, CT, F], f32, tag="x2")
                s5lt = spk.tile([128, CT, F], f32, tag="s5lt")
                for m in range(CT):
                    ps = ps_p.tile([128, F], f32, tag="p")
                    for k in range(CT):
                        nc.tensor.matmul(
                            ps, t_wp[:, k, m * 128:(m + 1) * 128].bitcast(f32r),
                            z[:, k, :].bitcast(f32r),
                            start=(k == 0), stop=(k == CT - 1))
                    nc.vector.scalar_tensor_tensor(
                        x2[:, m, :], ps, bias_ap(BP + m), xt[:, m, :],
                        alu.add, alu.add)
                    if t == 0:
                        u5 = x2[:, m, :]
                    else:
                        u5 = up.tile([128, F], f32, tag="u")
                        gps_add(u5, x2[:, m, :], w5s[:, m, :])
                    nc.vector.tensor_scalar(s5lt[:, m, :].bitcast(f32r), u5,
                                            2.0, 0.5, alu.is_lt, alu.mult)
                    if t < T - 1:
                        gps_reset(w5s[:, m, :], u5, s5lt[:, m, :])

                if pending_f2c is not None:
                    pending_f2c()
                    pending_f2c = None

                # ===== fc1 (fp32r) + lif6, m-tiles in pairs (wide DVE ops) =====
                s6lt = spk6.tile([128, HT, F], bf16, tag="s6lt")
                for j in range(HT // 2):
                    u_pair = up2.tile([128, 2 * F], f32, tag="u2")
                    for h in range(2):
                        m = 2 * j + h
                        ps = ps_f1.tile([128, F], f32, tag="f1")
                        for k in range(CT):
                            nc.tensor.matmul(
                                ps, t_w1[:, k, m * 128:(m + 1) * 128].bitcast(f32r),
                                s5lt[:, k, :].bitcast(f32r),
                                start=(k == 0), stop=(k == CT - 1))
                        uh = u_pair[:, h * F:(h + 1) * F]
                        if t == 0:
                            nc.scalar.activation(uh, ps, AF.Identity,
                                                 bias=bias_ap(B1 + m), scale=1.0)
                        else:
                            nc.vector.scalar_tensor_tensor(
                                uh, ps, bias_ap(B1 + m), w6s[:, m, :],
                                alu.add, alu.add)
                    s6_pair = s6lt[:, 2 * j:2 * j + 2, :].rearrange(
                        "p a b -> p (a b)")
                    eng6 = nc.gpsimd if t == T - 1 else nc.vector
                    eng6.tensor_scalar(s6_pair, u_pair, 2.0, 0.5,
                                       alu.is_lt, alu.mult)
                    if t < T - 1:
                        w6_pair = w6s[:, 2 * j:2 * j + 2, :].rearrange(
                            "p a b -> p (a b)")
                        gps_reset(w6_pair, u_pair, s6_pair)

                # ===== prefetch + lif1 of t+1 (fills DVE during fc2's PE run) =====
                if t + 1 < T:
                    xt_next = xin.tile([128, CT, F], f32, tag="xt")
                    xts[t + 1] = xt_next
                    nc.sync.dma_start(out=xt_next, in_=d_x[t + 1])
                    xlt_cur = stage0(t + 1, xts[t + 1])

                # ===== fc2 (1-pass bf16) + output residual, deferred =====
                def make_f2(t, s6lt, x2, ms):
                    def emit():
                        for m in ms:
                            ps = ps_f2.tile([128, F], f32, tag="f2", name=f"psf2_{t}_{m}")
                            for k in range(HT):
                                nc.tensor.matmul(
                                    ps, t_w2[:, k, m * 128:(m + 1) * 128],
                                    s6lt[:, k, :],
                                    start=(k == 0), stop=(k == HT - 1))
                            ot = outp.tile([128, F], f32, tag="ot", name=f"ot_{t}_{m}")
                            nc.vector.scalar_tensor_tensor(
                                ot, ps, bias_ap(B2 + m), x2[:, m, :],
                                alu.add, alu.add)
                            nc.sync.dma_start(out=d_out[t, :, m, :], in_=ot)
                    return emit
                pending_f2 = make_f2(t, s6lt, x2, [0])
                pending_f2b = make_f2(t, s6lt, x2, [1])
                pending_f2c = make_f2(t, s6lt, x2, [2])

            if pending_f2 is not None:
                pending_f2()
            if pending_f2b is not None:
                pending_f2b()
            if pending_f2c is not None:
                pending_f2c()

    nc.compile()
    return nc


def _host_arrays(x, qw, bn_q, kw, bn_k, pw, pb, bn_p, f1w, f1b, bn1,
                 f2w, f2b, bn2):
    """Returns (shared weight/bias arrays dict, per-core x shards list)."""
    (wq_eff, bq_eff, wk_eff, bk_eff, wp_eff, bp_eff,
     w1_eff, b1_eff, w2_eff, b2_eff) = _fold_host(
        qw, bn_q, kw, bn_k, pw, pb, bn_p, f1w, f1b, bn1, f2w, f2b, bn2)

    a_wq = _pm(wq_eff.T, np.float32)
    a_wk = _pm(wk_eff.T, np.float32)
    a_wp = _pm(wp_eff.T, np.float32)
    a_w1 = _pm(w1_eff.T, np.float32)
    a_w2 = _pm(w2_eff.T.astype(np.float32).astype(ml_dtypes.bfloat16),
               ml_dtypes.bfloat16)

    msk = np.zeros((C, NH), dtype=ml_dtypes.bfloat16)
    for c in range(C):
        msk[c, c // DH] = 1.0
    a_msk = np.ascontiguousarray(
        msk.reshape(CT, 128, NH).transpose(1, 0, 2))
    a_ee = np.ascontiguousarray(msk.T)

    bias = np.zeros((128, NB), dtype=np.float32)
    for vec, ofs, nt in ((bq_eff, 0, CT), (bk_eff, CT, CT), (bp_eff, 2 * CT, CT),
                         (b1_eff, 3 * CT, HT), (b2_eff, 3 * CT + HT, CT)):
        bias[:, ofs:ofs + nt] = vec.astype(np.float32).reshape(nt, 128).T
    bias[:, 3 * CT + HT + CT] = float(DH)

    shared = {
        "wq": a_wq, "wk": a_wk, "wp": a_wp, "w1": a_w1, "w2": a_w2,
        "msk": a_msk, "ee": a_ee, "bias": bias,
    }
    x = np.asarray(x, dtype=np.float32)
    shards = []
    for c in range(NCORES):
        xs = x[:, c * BL:(c + 1) * BL]                      # (T,BL,C,H,W)
        xs = xs.reshape(T, BL, C, N).transpose(0, 2, 1, 3)  # (T,C,BL,N)
        xs = xs.reshape(T, CT, 128, F).transpose(0, 2, 1, 3)  # (T,128,CT,F)
        shards.append(np.ascontiguousarray(xs))
    return shared, shards


def _unshard_out(core_out):
    """(T,128,CT,F) core output -> (T,BL,C,H,W)."""
    o = core_out.transpose(0, 2, 1, 3)                  # (T,CT,128,F)
    o = o.reshape(T, C, BL, N).transpose(0, 2, 1, 3)    # (T,BL,C,N)
    return o.reshape(T, BL, C, H, W_SP)


def kernel(x, qw, bn_q, kw, bn_k, pw, pb, bn_p, f1w, f1b, bn1, f2w, f2b, bn2):
    from concourse.bass_utils import run_bass_kernel_spmd

    shared, shards = _host_arrays(x, qw, bn_q, kw, bn_k, pw, pb, bn_p,
                                  f1w, f1b, bn1, f2w, f2b, bn2)
    if "nc" not in _COMPILED:
        _COMPILED["nc"] = _build_module()
    nc = _COMPILED["nc"]

    in_maps = [{**shared, "x": shards[c]} for c in range(NCORES)]
    res = run_bass_kernel_spmd(nc, in_maps, core_ids=list(range(NCORES)))
    _COMPILED["last_results"] = res

    out = np.empty((T, B, C, H, W_SP), dtype=np.float32)
    for c in range(NCORES):
        out[:, c * BL:(c + 1) * BL] = _unshard_out(res.results[c]["out"])
    return out
